# revision 3
# baseline (speedup 1.0000x reference)
"""LocalLinear (per-position dense) Trainium2 kernel (optimized).

out[b, f, l] = sum_k xpad[b, f+k] * w[f, k, l] + bias[f, l]
  x: [256, 4096] f32, w: [4096, 64, 32] f32 -> out: [256, 4096, 32] f32

Fold-sharded across 8 cores (512 folds each). v8 uses 32-fold groups:
each group's einsum is a [96u x 128b]^T @ [96u x 1024] matmul against
a banded weight block (96 = 32 folds + 63 taps window). vs 64-fold
groups this cuts banded-weight HBM bytes 4.19 -> 3.07 MB/core (band
inflation (G+64)/64 shrinks with G; SBUF-port efficiency of 96/128
doesn't matter because HBM, not the port fabric, is the bottleneck).
All device I/O is bf16 (rel err ~4e-3 vs the 2e-2 gate); f32 upcast
on host.
"""
import sys

if '/opt/trn_rl_repo' not in sys.path:
    sys.path.insert(0, '/opt/trn_rl_repo')

import numpy as np
import ml_dtypes

import concourse.bass as bass
import concourse.tile as tile
from concourse import bacc, mybir
from concourse import bass_utils

B = 256
IN = 4096
KS = 64
L = 32
FOLD = 4096
NCORES = 8
FPC = FOLD // NCORES          # folds per core = 512
G32 = 32                      # folds per group
JPC = FPC // G32              # groups per core = 16
CTR = G32 + KS                # contraction rows per group = 96
CW = G32 * L                  # free-dim columns per group = 1024

BF16 = mybir.dt.bfloat16
NPBF = ml_dtypes.bfloat16
_cache = {}


def _build_nc():
    nc = bacc.Bacc("TRN2", target_bir_lowering=False, debug=False)
    # xg[p, 256j+b] = xpad[b, 512c + 32j + p], p in [0,96)
    xg_d = nc.dram_tensor("xg", [CTR, JPC * B], BF16, kind="ExternalInput")
    wb_d = nc.dram_tensor("wb", [JPC, CTR, CW], BF16, kind="ExternalInput")
    out_d = nc.dram_tensor("out", [B, FPC, L], BF16, kind="ExternalOutput")

    with tile.TileContext(nc) as tc:
        with (
            tc.tile_pool(name="xg", bufs=1) as xg_pool,
            tc.tile_pool(name="wb", bufs=JPC) as wb_pool,
            tc.tile_pool(name="ps", bufs=4, space="PSUM") as ps_pool,
            tc.tile_pool(name="ob", bufs=3) as ob_pool,
        ):
            xg_t = xg_pool.tile([CTR, JPC * B], BF16)
            nc.sync.dma_start(xg_t[:], xg_d[:])
            wb_ts = []
            for j in range(JPC):
                wb_t = wb_pool.tile([CTR, CW], BF16, tag="wb")
                nc.sync.dma_start(wb_t[:], wb_d[j])
                wb_ts.append(wb_t)

            cp = 0  # alternate PSUM->SBUF casts across DVE/ACT
            for h in range(2):
                for gp in range(JPC // 4):
                    ob = ob_pool.tile([128, 4, G32, L], BF16)
                    for s in range(4):
                        j = 4 * gp + s
                        lhsT = xg_t[:, B * j + 128 * h: B * j + 128 * h + 128]
                        ps = ps_pool.tile([128, CW], mybir.dt.float32)
                        for jj in range(2):
                            nc.tensor.matmul(
                                ps[:, 512 * jj: 512 * jj + 512], lhsT,
                                wb_ts[j][:, 512 * jj: 512 * jj + 512])
                        dst = ob[:, s, :, :]
                        if cp % 2 == 0:
                            nc.vector.tensor_copy(dst, ps[:])
                        else:
                            nc.scalar.copy(dst, ps[:])
                        cp += 1
                    nc.sync.dma_start(
                        out_d[128 * h: 128 * h + 128,
                              128 * gp: 128 * gp + 128, :],
                        ob[:],
                    )
    nc.compile()
    return nc


def _host_prep(x, weight):
    # xt: padded transpose of x, [4160, 256], bf16
    xt = np.zeros((FOLD + KS, B), NPBF)
    xt[:IN] = np.ascontiguousarray(x.T)
    # banded weights per 32-fold group: W32[J, r+k, r*L+l] = w[32J+r, k, l]
    GJ = FOLD // G32
    W32 = np.zeros((GJ, CTR, G32, L), NPBF)
    wg = weight.astype(NPBF).reshape(GJ, G32, KS, L)
    for r in range(G32):
        W32[:, r:r + KS, r, :] = wg[:, r, :, :]
    W32 = W32.reshape(GJ, CTR, CW)
    return xt, W32


def _in_maps(x, weight):
    xt, W32 = _host_prep(x, weight)
    in_maps = []
    for c in range(NCORES):
        base = FPC * c
        xg = np.stack(
            [xt[base + G32 * j: base + G32 * j + CTR] for j in range(JPC)],
            axis=1)  # [96, JPC, 256]
        in_maps.append({
            "xg": np.ascontiguousarray(xg.reshape(CTR, JPC * B)),
            "wb": np.ascontiguousarray(W32[JPC * c: JPC * c + JPC]),
        })
    return in_maps


def kernel(x, weight, bias):
    x = np.asarray(x, dtype=np.float32)
    weight = np.asarray(weight, dtype=np.float32)
    bias = np.asarray(bias, dtype=np.float32)

    if 'nc' not in _cache:
        _cache['nc'] = _build_nc()
    nc = _cache['nc']

    in_maps = _in_maps(x, weight)
    res = bass_utils.run_bass_kernel_spmd(
        nc, in_maps, core_ids=list(range(NCORES)), trace=False)

    out = np.concatenate(
        [res.results[c]["out"].astype(np.float32) for c in range(NCORES)],
        axis=1)
    if np.any(bias):
        out = out + bias[None, :, :]
    return out
